# revision 12
# baseline (speedup 1.0000x reference)
"""Trainium2 Bass kernel for the PolymorphicSNN forward pass.

Strategy: data-parallel over batch B across 8 NeuronCores (B=2048 -> 256/core).
Params replicated. The two batch-global reductions (mode-usage average for
microsleep masking, and the global inhibition mean) are folded into a single
[9,72] partial-sum matrix per core that is combined with one small AllReduce
mid-kernel; everything else is local.

Math notes (exact rewrites of the reference, no approximations):
  xl      = x @ W_lin.T + b_lin
  scores  = softmax_m(xl @ WselT + bsel)        (softmax without max-shift: logits ~ N(0,1))
  eq      = scores @ Wf  (per p);  new_x = xl + 0.5*k_p*dt^2*(eq - xl)
  spk     = new_x > 1;  q[p,b] = mean_n spk
  supp-sum f[p,b] = sum_m scores*w[p,m],  w = 1-0.9*[avg_B scores > thr]
  mixed   = spk * f;  means = f*q;  cf = coupling @ means
  inhibition = g * (sum_pb f*q)/(P*B)
             = g * (sum_p A_p - 0.9*sum_pm mask*C_pm)/(P*B)      with
               A_p = sum_b q,  C_pm = sum_b scores*q,  D_pm = sum_b scores
  final   = (xl>1) + mean_p mixed + mean_p cf - inhibition
A, C, D are local partial sums collected by one [128,9]x[128,72] matmul per
b-tile (lhsT = [q_counts | ones], rhs = [scores | q_counts]) and AllReduced.
"""

import numpy as np

import concourse.bass as bass
import concourse.bacc as bacc
import concourse.mybir as mybir
import concourse.tile as tile
from concourse import bass_utils
from concourse.masks import make_identity

F32 = mybir.dt.float32
BF16 = mybir.dt.bfloat16
AX = mybir.AxisListType
OP = mybir.AluOpType
AF = mybir.ActivationFunctionType

B, N, P, M = 2048, 1024, 8, 8
PM = P * M  # 64
NC = 8
BL = B // NC          # 256 rows per core
NBT = BL // 128       # 2 b-tiles per core
NKT = N // 128        # 8 k-tiles
NNH = N // 512        # 2 free-dim halves
DT = 0.01
THRESHOLD = 1.0


def _emit(nc, use_bias, use_bsel):
    x_d = nc.dram_tensor("x", [BL, N], F32, kind="ExternalInput")
    wlT_d = nc.dram_tensor("wlT", [N, N], F32, kind="ExternalInput")
    wselT_d = nc.dram_tensor("wselT", [N, PM], F32, kind="ExternalInput")
    wf_d = nc.dram_tensor("wf", [PM, N], F32, kind="ExternalInput")
    kk_d = nc.dram_tensor("kk", [1, P], F32, kind="ExternalInput")       # 0.5*dt^2*spring_k
    msthr_d = nc.dram_tensor("msthr", [1, P], F32, kind="ExternalInput")
    coupTs_d = nc.dram_tensor("coupTs", [P, P], F32, kind="ExternalInput")  # coupling.T / N
    ginh_d = nc.dram_tensor("ginh", [1, 1], F32, kind="ExternalInput")
    blin_d = nc.dram_tensor("blin", [1, N], F32, kind="ExternalInput") if use_bias else None
    bsel_d = nc.dram_tensor("bselv", [1, PM], F32, kind="ExternalInput") if use_bsel else None

    out_final = nc.dram_tensor("out_final", [BL, N], F32, kind="ExternalOutput")
    out_regmem = nc.dram_tensor("out_regmem", [BL, N], F32, kind="ExternalOutput")
    out_newx = nc.dram_tensor("out_newx", [P, BL, N], F32, kind="ExternalOutput")

    with tile.TileContext(nc) as tc:
        _body(nc, tc, locals())


def _body(nc, tc, t):
    from contextlib import ExitStack
    ctx = ExitStack()
    with ctx:
        const = ctx.enter_context(tc.tile_pool(name="const", bufs=1))
        work = ctx.enter_context(tc.tile_pool(name="work", bufs=1))
        trans = ctx.enter_context(tc.tile_pool(name="trans", bufs=4))
        # PSUM budget is 8 banks: tp(2 x [128,128]) + mm(2 x [128,512])
        # + eq(2 x [128,512]) + sm(1 x [128,64]) + dca(1 x [9,72]) = 8
        ps_tp = ctx.enter_context(tc.tile_pool(name="ps_tp", bufs=2, space="PSUM"))
        ps_mm = ctx.enter_context(tc.tile_pool(name="ps_mm", bufs=2, space="PSUM"))
        ps_eq = ctx.enter_context(tc.tile_pool(name="ps_eq", bufs=2, space="PSUM"))
        ps_sm = ctx.enter_context(tc.tile_pool(name="ps_sm", bufs=1, space="PSUM"))

        # ---------------- constants / params in SBUF ----------------
        id128 = const.tile([128, 128], F32)
        make_identity(nc, id128[:])

        wl = const.tile([128, NKT, N], F32)          # W_lin.T as k-tiles
        for kt in range(NKT):
            nc.sync.dma_start(wl[:, kt, :], t["wlT_d"].ap()[kt * 128:(kt + 1) * 128, :])
        wsel = const.tile([128, NKT, PM], F32)
        nc.sync.dma_start(wsel[:], t["wselT_d"].ap().rearrange("(kt k) m -> k kt m", k=128))
        # per-p force-weight tiles, each based at partition 0 (PE requires
        # matmul operands to start at partition 0/32/64)
        wf_bf = []
        for p in range(P):
            w32 = work.tile([M, N], F32, tag=f"wf32_{p}", name=f"wf32_{p}")
            nc.sync.dma_start(w32[:], t["wf_d"].ap()[p * M:(p + 1) * M, :])
            wbf = const.tile([M, N], BF16, name=f"wfbf_{p}")
            nc.vector.tensor_copy(wbf[:], w32[:])
            wf_bf.append(wbf)
        kkb = const.tile([128, P], F32)
        nc.sync.dma_start(kkb[:], t["kk_d"].ap().to_broadcast([128, P]))
        msthr_sb = const.tile([1, P], F32)
        nc.sync.dma_start(msthr_sb[:], t["msthr_d"].ap())
        coupTs_sb = const.tile([P, P], F32)
        nc.sync.dma_start(coupTs_sb[:], t["coupTs_d"].ap())
        ginh_sb = const.tile([1, 1], F32)
        nc.sync.dma_start(ginh_sb[:], t["ginh_d"].ap())
        if t["use_bias"]:
            blin_sb = const.tile([1, N], F32)
            nc.sync.dma_start(blin_sb[:], t["blin_d"].ap())
        if t["use_bsel"]:
            bsel_sb = const.tile([1, PM], F32)
            nc.sync.dma_start(bsel_sb[:], t["bsel_d"].ap())
        if t["use_bias"] or t["use_bsel"]:
            ones1 = const.tile([1, 128], F32)
            nc.vector.memset(ones1[:], 1.0)

        # ---------------- load x, transpose to xT ----------------
        xb = []
        for bt in range(NBT):
            xt_ = work.tile([128, N], F32, tag=f"xb{bt}", name=f"xb{bt}")
            nc.sync.dma_start(xt_[:], t["x_d"].ap()[bt * 128:(bt + 1) * 128, :])
            xb.append(xt_)
        xT = work.tile([128, NKT, NBT, 128], F32, tag="xT", name="xT")
        for bt in range(NBT):
            for kt in range(NKT):
                ps = ps_tp.tile([128, 128], F32, tag="tp", name="tp")
                nc.tensor.transpose(ps[:], xb[bt][:, kt * 128:(kt + 1) * 128], id128[:])
                nc.scalar.copy(xT[:, kt, bt, :], ps[:])

        # ---------------- main matmul: xl = x @ W_lin.T (+ b_lin) ----------------
        xl = [work.tile([128, N], F32, tag=f"xl{bt}", name=f"xl{bt}") for bt in range(NBT)]
        for bt in range(NBT):
            for nh in range(NNH):
                ps = ps_mm.tile([128, 512], F32, tag="mm", name="mm")
                for kt in range(NKT):
                    nc.tensor.matmul(ps[:], xT[:, kt, bt, :], wl[:, kt, nh * 512:(nh + 1) * 512],
                                     start=(kt == 0), stop=(kt == NKT - 1 and not t["use_bias"]))
                if t["use_bias"]:
                    nc.tensor.matmul(ps[:], ones1[:], blin_sb[:, nh * 512:(nh + 1) * 512],
                                     start=False, stop=True)
                nc.scalar.copy(xl[bt][:, nh * 512:(nh + 1) * 512], ps[:])

        # reg_mem output + reg_out spikes (gpsimd to offload DVE)
        rg = []
        for bt in range(NBT):
            nc.sync.dma_start(t["out_regmem"].ap()[bt * 128:(bt + 1) * 128, :], xl[bt][:])
            r = work.tile([128, N], BF16, tag=f"rg{bt}", name=f"rg{bt}")
            nc.gpsimd.tensor_scalar(r[:], xl[bt][:], THRESHOLD, None, OP.is_gt)
            rg.append(r)

        # ---------------- transpose xl -> xlT ----------------
        xlT = work.tile([128, NKT, NBT, 128], F32, tag="xlT", name="xlT")
        for bt in range(NBT):
            for kt in range(NKT):
                ps = ps_tp.tile([128, 128], F32, tag="tp", name="tp")
                nc.tensor.transpose(ps[:], xl[bt][:, kt * 128:(kt + 1) * 128], id128[:])
                nc.scalar.copy(xlT[:, kt, bt, :], ps[:])

        # ---------------- selector scores ----------------
        scores = []
        scT = [[work.tile([M, 128], BF16, tag=f"scT{bt}_{p}", name=f"scT{bt}_{p}")
                for p in range(P)] for bt in range(NBT)]
        for bt in range(NBT):
            ps = ps_sm.tile([128, PM], F32, tag="sm", name="sm")
            for kt in range(NKT):
                nc.tensor.matmul(ps[:], xlT[:, kt, bt, :], wsel[:, kt, :],
                                 start=(kt == 0), stop=(kt == NKT - 1 and not t["use_bsel"]))
            if t["use_bsel"]:
                nc.tensor.matmul(ps[:], ones1[:], bsel_sb[:], start=False, stop=True)
            E = work.tile([128, PM], F32, tag=f"E{bt}", name=f"E{bt}")
            nc.scalar.activation(E[:], ps[:], AF.Exp)
            S = work.tile([128, P], F32, tag=f"S{bt}", name=f"S{bt}")
            nc.vector.tensor_reduce(S[:], E[:].rearrange("q (p m) -> q p m", m=M), AX.X, OP.add)
            R = work.tile([128, P], F32, tag=f"R{bt}", name=f"R{bt}")
            nc.vector.reciprocal(R[:], S[:])
            sc = work.tile([128, PM], F32, tag=f"sc{bt}", name=f"sc{bt}")
            nc.vector.tensor_tensor(
                sc[:].rearrange("q (p m) -> q p m", m=M),
                E[:].rearrange("q (p m) -> q p m", m=M),
                R[:, :, None].broadcast_to([128, P, M]), OP.mult)
            scores.append(sc)
            # transpose scores per-p for the eq matmuls (each lands at
            # partition 0; engines can't read PSUM at unaligned bases)
            for p in range(P):
                ps2 = ps_tp.tile([128, 128], F32, tag="tp", name="tp")
                nc.tensor.transpose(ps2[:M, :], sc[:, p * M:(p + 1) * M], id128[:])
                nc.scalar.copy(scT[bt][p][:], ps2[:M, :])

        # ---------------- per-p: eq, new_x, spk, q ----------------
        spk = [[work.tile([128, N], BF16, tag=f"spk{bt}_{p}", name=f"spk{bt}_{p}") for p in range(P)]
               for bt in range(NBT)]
        qacc = [[work.tile([128, P], F32, tag=f"qacc{bt}_{nh}", name=f"qacc{bt}_{nh}") for nh in range(NNH)]
                for bt in range(NBT)]
        for bt in range(NBT):
            for p in range(P):
                for nh in range(NNH):
                    sl = slice(nh * 512, (nh + 1) * 512)
                    ps = ps_eq.tile([128, 512], F32, tag="eq", name="eq")
                    nc.tensor.matmul(ps[:], scT[bt][p][:],
                                     wf_bf[p][:, sl], start=True, stop=True)
                    d = trans.tile([128, 512], F32, tag="d", name="d")
                    nc.vector.tensor_sub(d[:], ps[:], xl[bt][:, sl])
                    nx = trans.tile([128, 512], F32, tag="nx", name="nx")
                    nc.vector.scalar_tensor_tensor(nx[:], d[:], kkb[:, p:p + 1], xl[bt][:, sl],
                                                   OP.mult, OP.add)
                    nc.sync.dma_start(t["out_newx"].ap()[p, bt * 128:(bt + 1) * 128, sl], nx[:])
                    # accum_out (TensorScalarPtrReduce) requires both ops; op1 is a no-op
                    nc.vector.tensor_scalar(spk[bt][p][:, sl], nx[:], THRESHOLD, 0.0,
                                            OP.is_gt, OP.add,
                                            accum_out=qacc[bt][nh][:, p:p + 1])

        # ---------------- partial sums matmul + AllReduce ----------------
        Qc, lhs_s, rhs_s = [], [], []
        for bt in range(NBT):
            q = work.tile([128, P], F32, tag=f"Qc{bt}", name=f"Qc{bt}")
            nc.vector.tensor_add(q[:], qacc[bt][0][:], qacc[bt][1][:])
            Qc.append(q)
            ls = work.tile([128, P + 1], F32, tag=f"ls{bt}", name=f"ls{bt}")
            nc.vector.tensor_copy(ls[:, 0:P], q[:])
            nc.vector.memset(ls[:, P:P + 1], 1.0)
            lhs_s.append(ls)
            rs = work.tile([128, PM + P], F32, tag=f"rs{bt}", name=f"rs{bt}")
            nc.vector.tensor_copy(rs[:, 0:PM], scores[bt][:])
            nc.vector.tensor_copy(rs[:, PM:PM + P], q[:])
            rhs_s.append(rs)
        ps_dca = ps_sm.tile([P + 1, PM + P], F32, tag="dca", name="dca")
        for bt in range(NBT):
            nc.tensor.matmul(ps_dca[:], lhs_s[bt][:], rhs_s[bt][:],
                             start=(bt == 0), stop=(bt == NBT - 1))
        dca_sb = work.tile([P + 1, PM + P], F32, tag="dca_sb", name="dca_sb")
        nc.scalar.copy(dca_sb[:], ps_dca[:])

        cc_in, _ = tc.tile([P + 1, PM + P], F32, space="DRAM", name="cc_in")
        cc_out, _ = tc.tile([P + 1, PM + P], F32, space="DRAM", addr_space="Shared", name="cc_out")
        nc.sync.dma_start(cc_in[:], dca_sb[:])
        nc.gpsimd.collective_compute(
            "AllReduce", OP.add, replica_groups=[list(range(NC))],
            ins=[cc_in[:].opt()], outs=[cc_out[:].opt()])

        # ---------------- post-reduce small math (partition 0) ----------------
        Dv = work.tile([1, PM], F32, tag="Dv", name="Dv")
        nc.sync.dma_start(Dv[:], cc_out[P:P + 1, 0:PM])
        Av = work.tile([1, P], F32, tag="Av", name="Av")
        nc.sync.dma_start(Av[:], cc_out[P:P + 1, PM:PM + P])
        Cv = work.tile([1, PM], F32, tag="Cv", name="Cv")
        cc_ap = cc_out[:]
        diag = bass.AP(tensor=cc_ap.tensor, offset=cc_ap.offset,
                       ap=[[0, 1], [PM + P + M, P], [1, M]])
        nc.sync.dma_start(Cv[:].rearrange("a (p m) -> a p m", m=M), diag)

        avg = work.tile([1, PM], F32, tag="avg", name="avg")
        nc.vector.tensor_scalar(avg[:], Dv[:], 1.0 / B, None, OP.mult)
        mask = work.tile([1, PM], F32, tag="mask", name="mask")
        nc.vector.tensor_tensor(
            mask[:].rearrange("a (p m) -> a p m", m=M),
            avg[:].rearrange("a (p m) -> a p m", m=M),
            msthr_sb[:, :, None].broadcast_to([1, P, M]), OP.is_gt)
        wv = work.tile([1, PM], F32, tag="wv", name="wv")
        nc.vector.tensor_scalar(wv[:], mask[:], -0.9, 1.0, OP.mult, OP.add)
        tv = work.tile([1, PM], F32, tag="tv", name="tv")
        nc.vector.tensor_mul(tv[:], mask[:], Cv[:])
        tsum = work.tile([1, 1], F32, tag="tsum", name="tsum")
        nc.vector.tensor_reduce(tsum[:], tv[:], AX.X, OP.add)
        asum = work.tile([1, 1], F32, tag="asum", name="asum")
        nc.vector.tensor_reduce(asum[:], Av[:], AX.X, OP.add)
        u = work.tile([1, 1], F32, tag="u", name="u")
        nc.vector.scalar_tensor_tensor(u[:], tsum[:], -0.9, asum[:], OP.mult, OP.add)
        nc.vector.tensor_mul(u[:], u[:], ginh_sb[:])
        inh = work.tile([1, 1], F32, tag="inh", name="inh")
        nc.vector.tensor_scalar(inh[:], u[:], 1.0 / (1024.0 * P * B), None, OP.mult)

        # broadcast w and inh to all partitions via DRAM round-trip
        wv_d, _ = tc.tile([1, PM], F32, space="DRAM", name="wv_d")
        inh_d, _ = tc.tile([1, 1], F32, space="DRAM", name="inh_d")
        nc.sync.dma_start(wv_d[:], wv[:])
        nc.sync.dma_start(inh_d[:], inh[:])
        wbc = work.tile([128, PM], F32, tag="wbc", name="wbc")
        nc.sync.dma_start(wbc[:], wv_d[:].to_broadcast([128, PM]))
        inhbc = work.tile([128, 1], F32, tag="inhbc", name="inhbc")
        nc.sync.dma_start(inhbc[:], inh_d[:].to_broadcast([128, 1]))

        # ---------------- f, coupling, final combine ----------------
        for bt in range(NBT):
            u1 = work.tile([128, PM], F32, tag=f"u1{bt}", name=f"u1{bt}")
            nc.vector.tensor_mul(u1[:], scores[bt][:], wbc[:])
            f = work.tile([128, P], F32, tag=f"f{bt}", name=f"f{bt}")
            nc.vector.tensor_reduce(f[:], u1[:].rearrange("q (p m) -> q p m", m=M), AX.X, OP.add)
            mt = work.tile([128, P], F32, tag=f"mt{bt}", name=f"mt{bt}")
            nc.vector.tensor_mul(mt[:], f[:], Qc[bt][:])
            ps_t = ps_tp.tile([128, 128], F32, tag="tp", name="tp")
            nc.tensor.transpose(ps_t[:P, :], mt[:], id128[:])
            mtT = work.tile([P, 128], F32, tag=f"mtT{bt}", name=f"mtT{bt}")
            nc.scalar.copy(mtT[:], ps_t[:P, :])
            ps_cf_t = ps_eq.tile([128, 512], F32, tag="eq", name="eq")
            nc.tensor.matmul(ps_cf_t[:, 0:P], mtT[:], coupTs_sb[:], start=True, stop=True)
            csum = work.tile([128, 1], F32, tag=f"csum{bt}", name=f"csum{bt}")
            nc.vector.tensor_reduce(csum[:], ps_cf_t[:, 0:P], AX.X, OP.add)
            off = work.tile([128, 1], F32, tag=f"off{bt}", name=f"off{bt}")
            nc.vector.scalar_tensor_tensor(off[:], csum[:], 1.0 / P, inhbc[:],
                                           OP.mult, OP.subtract)
            for nh in range(NNH):
                sl = slice(nh * 512, (nh + 1) * 512)
                acc = trans.tile([128, 512], F32, tag="acc", name="acc")
                nc.vector.tensor_scalar(acc[:], spk[bt][0][:, sl], f[:, 0:1], None, OP.mult)
                for p in range(1, P):
                    nc.vector.scalar_tensor_tensor(acc[:], spk[bt][p][:, sl], f[:, p:p + 1],
                                                   acc[:], OP.mult, OP.add)
                t2 = trans.tile([128, 512], F32, tag="t2", name="t2")
                nc.vector.tensor_scalar(t2[:], acc[:], 1.0 / P, off[:], OP.mult, OP.add)
                fin = trans.tile([128, 512], F32, tag="fin", name="fin")
                nc.vector.tensor_tensor(fin[:], t2[:], rg[bt][:, sl], OP.add)
                nc.sync.dma_start(t["out_final"].ap()[bt * 128:(bt + 1) * 128, sl], fin[:])


_CACHE = {}


def _get_compiled(use_bias, use_bsel):
    key = (use_bias, use_bsel)
    if key not in _CACHE:
        nc = bacc.Bacc("TRN2", target_bir_lowering=False, debug=False,
                       enable_asserts=False, num_devices=NC)
        _emit(nc, use_bias, use_bsel)
        nc.compile()
        _CACHE[key] = nc
    return _CACHE[key]


def _prep_inputs(x, W_lin, b_lin, Wsel, bsel, Wf, spring_k, damping_c, ms_thr,
                 coupling, global_inh):
    f32 = lambda a: np.ascontiguousarray(np.asarray(a, dtype=np.float32))
    x = f32(x)
    shared = {
        "wlT": f32(np.asarray(W_lin, dtype=np.float32).T),
        "wselT": f32(np.asarray(Wsel, dtype=np.float32).reshape(PM, N).T),
        "wf": f32(np.asarray(Wf, dtype=np.float32).reshape(PM, N)),
        "kk": f32(np.asarray(spring_k) * (0.5 * DT * DT)).reshape(1, P),
        "msthr": f32(ms_thr).reshape(1, P),
        "coupTs": f32(np.asarray(coupling, dtype=np.float32).T / 1024.0),
        "ginh": f32(global_inh).reshape(1, 1),
    }
    use_bias = bool(np.any(np.asarray(b_lin)))
    use_bsel = bool(np.any(np.asarray(bsel)))
    if use_bias:
        shared["blin"] = f32(b_lin).reshape(1, N)
    if use_bsel:
        shared["bselv"] = f32(bsel).reshape(1, PM)
    in_maps = []
    for c in range(NC):
        m = dict(shared)
        m["x"] = np.ascontiguousarray(x[c * BL:(c + 1) * BL])
        in_maps.append(m)
    return in_maps, use_bias, use_bsel


def kernel(**inputs):
    in_maps, use_bias, use_bsel = _prep_inputs(**inputs)
    nc = _get_compiled(use_bias, use_bsel)
    res = bass_utils.run_bass_kernel_spmd(nc, in_maps, core_ids=list(range(NC)))
    final = np.empty((B, N), np.float32)
    regmem = np.empty((B, N), np.float32)
    newx = np.empty((P, B, N), np.float32)
    for c in range(NC):
        r = res.results[c]
        final[c * BL:(c + 1) * BL] = r["out_final"]
        regmem[c * BL:(c + 1) * BL] = r["out_regmem"]
        newx[:, c * BL:(c + 1) * BL] = r["out_newx"]
    poly_mems = np.broadcast_to(newx[:, None], (P, M, B, N))
    return final, regmem, poly_mems


# revision 45
# speedup vs baseline: 909.4237x; 909.4237x over previous
"""Trainium2 Bass kernel for the PolymorphicSNN forward pass.

Strategy: data-parallel over batch B across 8 NeuronCores (B=2048 -> 256/core).
Params replicated. The two batch-global reductions (mode-usage average for
microsleep masking, and the global inhibition mean) are folded into a single
[9,72] partial-sum matrix per core that is combined with one small AllReduce
mid-kernel; everything else is local.

Math notes (exact rewrites of the reference, no approximations):
  xl      = x @ W_lin.T + b_lin
  scores  = softmax_m(xl @ WselT + bsel)        (softmax without max-shift: logits ~ N(0,1))
  eq      = scores @ Wf  (per p);  new_x = xl + 0.5*k_p*dt^2*(eq - xl)
  spk     = new_x > 1;  q[p,b] = mean_n spk
  supp-sum f[p,b] = sum_m scores*w[p,m],  w = 1-0.9*[avg_B scores > thr]
  mixed   = spk * f;  means = f*q;  cf = coupling @ means
  inhibition = g * (sum_pb f*q)/(P*B)
             = g * (sum_p A_p - 0.9*sum_pm mask*C_pm)/(P*B)      with
               A_p = sum_b q,  C_pm = sum_b scores*q,  D_pm = sum_b scores
  final   = (xl>1) + mean_p mixed + mean_p cf - inhibition
A, C, D are local partial sums collected by one [128,9]x[128,72] matmul per
b-tile (lhsT = [q_counts | ones], rhs = [scores | q_counts]) and AllReduced.
"""

import numpy as np

import concourse.bass as bass
import concourse.bacc as bacc
import concourse.mybir as mybir
import concourse.tile as tile
from concourse import bass_utils
from concourse.masks import make_identity

F32 = mybir.dt.float32
BF16 = mybir.dt.bfloat16
AX = mybir.AxisListType
OP = mybir.AluOpType
AF = mybir.ActivationFunctionType

# ablation/experiment switches (affect compiled program; cache key includes them)
OPTS = {"collective": True, "newx_dma": True, "regmem_dma": True, "final_dma": True,
        "slow": False, "repeat": 1}

B, N, P, M = 2048, 1024, 8, 8
PM = P * M  # 64
NC = 8
BL = B // NC          # 256 rows per core
NBT = BL // 128       # 2 b-tiles per core
NKT = N // 128        # 8 k-tiles
NNH = N // 512        # 2 free-dim halves
DT = 0.01
THRESHOLD = 1.0


def _emit(nc, use_bias, use_bsel):
    x_d = nc.dram_tensor("x", [BL, N], F32, kind="ExternalInput")
    wlT_d = nc.dram_tensor("wlT", [N, N], F32, kind="ExternalInput")
    wselT_d = nc.dram_tensor("wselT", [N, PM], F32, kind="ExternalInput")
    wf_d = nc.dram_tensor("wf", [PM, N], F32, kind="ExternalInput")
    kk_d = nc.dram_tensor("kk", [1, P], F32, kind="ExternalInput")       # 0.5*dt^2*spring_k
    msthr_d = nc.dram_tensor("msthr", [1, P], F32, kind="ExternalInput")
    coupTs_d = nc.dram_tensor("coupTs", [P, P], F32, kind="ExternalInput")  # coupling.T / N
    ginh_d = nc.dram_tensor("ginh", [1, 1], F32, kind="ExternalInput")
    blin_d = nc.dram_tensor("blin", [1, N], F32, kind="ExternalInput") if use_bias else None
    bsel_d = nc.dram_tensor("bselv", [1, PM], F32, kind="ExternalInput") if use_bsel else None

    out_final = nc.dram_tensor("out_final", [BL, N], F32, kind="ExternalOutput")
    out_regmem = nc.dram_tensor("out_regmem", [BL, N], F32, kind="ExternalOutput")
    out_newx = nc.dram_tensor("out_newx", [P, BL, N], F32, kind="ExternalOutput")
    out_dbg = nc.dram_tensor("out_dbg", [1, PM + 2], F32, kind="ExternalOutput")

    env = locals()
    with tile.TileContext(nc) as tc:
        for _rep in range(OPTS["repeat"]):
            _body(nc, tc, env)


def _body(nc, tc, t):
    from contextlib import ExitStack
    ctx = ExitStack()
    with ctx:
        const = ctx.enter_context(tc.tile_pool(name="const", bufs=1))
        work = ctx.enter_context(tc.tile_pool(name="work", bufs=1))
        trans = ctx.enter_context(tc.tile_pool(name="trans", bufs=4))
        # PSUM budget is 8 banks: tp(2 x [128,128]) + big(4 x [128,512],
        # shared by main-mm / eq / cf) + sm(1 x [128,64]) + dca(1 x [9,72]) = 8
        ps_tp = ctx.enter_context(tc.tile_pool(name="ps_tp", bufs=2, space="PSUM"))
        ps_big = ctx.enter_context(tc.tile_pool(name="ps_big", bufs=4, space="PSUM"))
        ps_mm = ps_big
        ps_eq = ps_big
        ps_sm = ctx.enter_context(tc.tile_pool(name="ps_sm", bufs=1, space="PSUM"))

        # ---------------- constants / params in SBUF ----------------
        id128 = const.tile([128, 128], F32)
        make_identity(nc, id128[:])
        # -I in bf16, used to accumulate -xl into the eq PSUM via the PE
        negI = const.tile([128, 128], BF16)
        nc.gpsimd.memset(negI[:], 0.0)
        nc.gpsimd.affine_select(out=negI[:], in_=negI[:], compare_op=OP.not_equal,
                                fill=-1.0, base=0, pattern=[[-1, 128]],
                                channel_multiplier=1)
        # +I in bf16: accumulates sum_p spk_p on the PE (spikes are 0/1, exact)
        posI = const.tile([128, 128], BF16)
        make_identity(nc, posI[:])

        # x loads FIRST (small, unblocks the PE transposes), then params;
        # wl k-tiles alternate across the two HWDGE rings (SP / Activation)
        xb = []
        for bt in range(NBT):
            xt_ = work.tile([128, N], F32, tag=f"xb{bt}", name=f"xb{bt}")
            nc.sync.dma_start(xt_[:], t["x_d"].ap()[bt * 128:(bt + 1) * 128, :])
            xb.append(xt_)
        wsel = const.tile([128, NKT, PM], F32)
        nc.scalar.dma_start(wsel[:], t["wselT_d"].ap().rearrange("(kt k) m -> k kt m", k=128))
        wl = const.tile([128, NKT, N], F32)          # W_lin.T as k-tiles
        for kt in range(NKT):
            eng = nc.sync if kt % 2 == 0 else nc.scalar
            eng.dma_start(wl[:, kt, :], t["wlT_d"].ap()[kt * 128:(kt + 1) * 128, :])
        # per-p force-weight tiles, each based at partition 0 (PE requires
        # matmul operands to start at partition 0/32/64)
        wf_bf = []
        for p in range(P):
            w32 = work.tile([M, N], F32, tag=f"wf32_{p}", name=f"wf32_{p}")
            nc.sync.dma_start(w32[:], t["wf_d"].ap()[p * M:(p + 1) * M, :])
            wbf = const.tile([M, N], BF16, name=f"wfbf_{p}")
            nc.vector.tensor_copy(wbf[:], w32[:])
            wf_bf.append(wbf)
        kkb = const.tile([128, P], F32)
        nc.sync.dma_start(kkb[:], t["kk_d"].ap().to_broadcast([128, P]))
        msthr_sb = const.tile([1, P], F32)
        nc.sync.dma_start(msthr_sb[:], t["msthr_d"].ap())
        coupTs_sb = const.tile([P, P], F32)
        nc.sync.dma_start(coupTs_sb[:], t["coupTs_d"].ap())
        ginh_sb = const.tile([1, 1], F32)
        nc.sync.dma_start(ginh_sb[:], t["ginh_d"].ap())
        if t["use_bias"]:
            blin_sb = const.tile([1, N], F32)
            nc.sync.dma_start(blin_sb[:], t["blin_d"].ap())
        if t["use_bsel"]:
            bsel_sb = const.tile([1, PM], F32)
            nc.sync.dma_start(bsel_sb[:], t["bsel_d"].ap())
        onesc = const.tile([1, 128], F32)
        nc.vector.memset(onesc[:], 1.0)
        ones1 = onesc

        # ---------------- transpose x to xT ----------------
        xT = work.tile([128, NKT, NBT, 128], F32, tag="xT", name="xT")
        for bt in range(NBT):
            for kt in range(NKT):
                ps = ps_tp.tile([128, 128], F32, tag="tp", name="tp")
                nc.tensor.transpose(ps[:], xb[bt][:, kt * 128:(kt + 1) * 128], id128[:])
                nc.scalar.copy(xT[:, kt, bt, :], ps[:])

        # ------- per-b-tile chain: mm -> xl -> scores -> eq/new_x/spk -------
        # Interleaved per bt so bt0's DVE/ACT work overlaps bt1's PE matmuls.
        xl = [work.tile([128, N], F32, tag=f"xl{bt}", name=f"xl{bt}") for bt in range(NBT)]
        xlT = work.tile([128, NKT, NBT, 128], F32, tag="xlT", name="xlT")
        rg, xl_bf, scores, Ss, Rs = [], [], [], [], []
        scT = [[work.tile([M, 128], BF16, tag=f"scT{bt}_{p}", name=f"scT{bt}_{p}")
                for p in range(P)] for bt in range(NBT)]
        spk = [[work.tile([128, N], BF16, tag=f"spk{bt}_{p}", name=f"spk{bt}_{p}")
                for p in range(P)] for bt in range(NBT)]
        qacc = [[work.tile([128, P], F32, tag=f"qacc{bt}_{nh}", name=f"qacc{bt}_{nh}")
                 for nh in range(NNH)] for bt in range(NBT)]
        for bt in range(NBT):
            # main matmul: xl = x @ W_lin.T (+ b_lin)
            for nh in range(NNH):
                ps = ps_mm.tile([128, 512], F32, tag="big", name="mm")
                for kt in range(NKT):
                    nc.tensor.matmul(ps[:], xT[:, kt, bt, :], wl[:, kt, nh * 512:(nh + 1) * 512],
                                     start=(kt == 0), stop=(kt == NKT - 1 and not t["use_bias"]))
                if t["use_bias"]:
                    nc.tensor.matmul(ps[:], ones1[:], blin_sb[:, nh * 512:(nh + 1) * 512],
                                     start=False, stop=True)
                nc.scalar.copy(xl[bt][:, nh * 512:(nh + 1) * 512], ps[:])

            # reg_mem output + reg_out spikes + bf16 copy of xl (gpsimd offload)
            if OPTS["regmem_dma"]:
                nc.sync.dma_start(t["out_regmem"].ap()[bt * 128:(bt + 1) * 128, :], xl[bt][:])
            r = work.tile([128, N], BF16, tag=f"rg{bt}", name=f"rg{bt}")
            nc.gpsimd.tensor_scalar(r[:], xl[bt][:], THRESHOLD, None, OP.is_gt)
            rg.append(r)
            xbf = work.tile([128, N], BF16, tag=f"xlbf{bt}", name=f"xlbf{bt}")
            nc.gpsimd.tensor_copy(xbf[:], xl[bt][:])
            xl_bf.append(xbf)

            # transpose xl -> xlT
            for kt in range(NKT):
                ps = ps_tp.tile([128, 128], F32, tag="tp", name="tp")
                nc.tensor.transpose(ps[:], xl[bt][:, kt * 128:(kt + 1) * 128], id128[:])
                nc.scalar.copy(xlT[:, kt, bt, :], ps[:])

            # selector scores
            ps = ps_sm.tile([128, PM], F32, tag="sm", name="sm")
            for kt in range(NKT):
                nc.tensor.matmul(ps[:], xlT[:, kt, bt, :], wsel[:, kt, :],
                                 start=(kt == 0), stop=(kt == NKT - 1 and not t["use_bsel"]))
            if t["use_bsel"]:
                nc.tensor.matmul(ps[:], ones1[:], bsel_sb[:], start=False, stop=True)
            E = work.tile([128, PM], F32, tag=f"E{bt}", name=f"E{bt}")
            nc.scalar.activation(E[:], ps[:], AF.Exp)
            S = work.tile([128, P], F32, tag=f"S{bt}", name=f"S{bt}")
            nc.vector.tensor_reduce(S[:], E[:].rearrange("q (p m) -> q p m", m=M), AX.X, OP.add)
            R = work.tile([128, P], F32, tag=f"R{bt}", name=f"R{bt}")
            nc.vector.reciprocal(R[:], S[:])
            Ss.append(S); Rs.append(R)
            sc = work.tile([128, PM], F32, tag=f"sc{bt}", name=f"sc{bt}")
            nc.vector.tensor_tensor(
                sc[:].rearrange("q (p m) -> q p m", m=M),
                E[:].rearrange("q (p m) -> q p m", m=M),
                R[:, :, None].broadcast_to([128, P, M]), OP.mult)
            scores.append(sc)
            # transpose scores per-p for the eq matmuls (each lands at
            # partition 0; engines can't read PSUM at unaligned bases)
            for p in range(P):
                ps2 = ps_tp.tile([128, 128], F32, tag="tp", name="tp")
                nc.tensor.transpose(ps2[:M, :], sc[:, p * M:(p + 1) * M], id128[:])
                nc.scalar.copy(scT[bt][p][:], ps2[:M, :])

            # per-p: eq, new_x, spk, q
            for p in range(P):
                nx = trans.tile([128, N], F32, tag="nx", name="nx", bufs=3)
                for nh in range(NNH):
                    sl = slice(nh * 512, (nh + 1) * 512)
                    ps = ps_eq.tile([128, 512], F32, tag="big", name="eq")
                    nc.tensor.matmul(ps[:], scT[bt][p][:],
                                     wf_bf[p][:, sl], start=True, stop=False)
                    # accumulate -xl (bf16; the 4e-5 spring scale makes the
                    # rounding negligible) so psum = eq - xl in one shot
                    nc.tensor.matmul(ps[:], negI[:], xl_bf[bt][:, sl],
                                     start=False, stop=True)
                    nc.vector.scalar_tensor_tensor(nx[:, sl], ps[:], kkb[:, p:p + 1],
                                                   xl[bt][:, sl], OP.mult, OP.add)
                    # accum_out (TensorScalarPtrReduce) requires both ops; op1 is a no-op
                    nc.vector.tensor_scalar(spk[bt][p][:, sl], nx[:, sl], THRESHOLD, 0.0,
                                            OP.is_gt, OP.add,
                                            accum_out=qacc[bt][nh][:, p:p + 1])
                # one full-row 512KB DMA per (p, b-tile): 4KB contiguous rows,
                # alternating between the two HWDGE rings (SP / Activation)
                if OPTS["newx_dma"]:
                    eng = nc.sync if p % 2 == 0 else nc.scalar
                    eng.dma_start(t["out_newx"].ap()[p, bt * 128:(bt + 1) * 128, :], nx[:])

        # ---------------- partial sums matmul + AllReduce ----------------
        # Also: f_fast = sum_m scores = S*R (exact when no mode sleeps), the
        # coupling matmul, and the spike-sum PE accumulation all run BEFORE
        # the collective so only a short scalar chain remains after it.
        Qc, lhs_s, rhs_s, fs, mts = [], [], [], [], []
        for bt in range(NBT):
            q = work.tile([128, P], F32, tag=f"Qc{bt}", name=f"Qc{bt}")
            nc.vector.tensor_add(q[:], qacc[bt][0][:], qacc[bt][1][:])
            Qc.append(q)
            ls = work.tile([128, P + 1], F32, tag=f"ls{bt}", name=f"ls{bt}")
            nc.scalar.copy(ls[:, 0:P], q[:])
            nc.vector.memset(ls[:, P:P + 1], 1.0)
            lhs_s.append(ls)
            rs = work.tile([128, PM + P], F32, tag=f"rs{bt}", name=f"rs{bt}")
            nc.scalar.copy(rs[:, 0:PM], scores[bt][:])
            nc.scalar.copy(rs[:, PM:PM + P], q[:])
            rhs_s.append(rs)
            if not OPTS["slow"]:
                f = work.tile([128, P], F32, tag=f"f{bt}", name=f"f{bt}")
                nc.vector.tensor_mul(f[:], Ss[bt][:], Rs[bt][:])
                fs.append(f)
                mt = work.tile([128, P], F32, tag=f"mt{bt}", name=f"mt{bt}")
                nc.vector.tensor_mul(mt[:], f[:], q[:])
                mts.append(mt)

        # sum_p spk on the PE (exact: spikes are 0/1). These 4 PSUM tiles
        # stay live across the AllReduce.
        accps = []
        if not OPTS["slow"]:
            for bt in range(NBT):
                row = []
                for nh in range(NNH):
                    sl = slice(nh * 512, (nh + 1) * 512)
                    ps_a = ps_big.tile([128, 512], F32, tag="big", name="accp")
                    for p in range(P):
                        nc.tensor.matmul(ps_a[:], posI[:], spk[bt][p][:, sl],
                                         start=(p == 0), stop=(p == P - 1))
                    row.append(ps_a)
                accps.append(row)

        # coupling matmul (pre-collective); -P*inh joins post-collective as a
        # K=1 rank-1 update into column P, so no partition-broadcast of inh
        # is needed on the fast path.
        if not OPTS["slow"]:
            cf_all = ps_sm.tile([128, PM], F32, tag="sm", name="cf_all")
            cfps = [cf_all[:, bt * (P + 1):(bt + 1) * (P + 1)] for bt in range(NBT)]
            for bt in range(NBT):
                ps_t = ps_tp.tile([128, 128], F32, tag="tp", name="tp")
                nc.tensor.transpose(ps_t[:P, :], mts[bt][:], id128[:])
                mtT = work.tile([P, 128], F32, tag=f"mtT{bt}", name=f"mtT{bt}")
                nc.scalar.copy(mtT[:], ps_t[:P, :])
                nc.tensor.matmul(cfps[bt][:, 0:P], mtT[:], coupTs_sb[:],
                                 start=True, stop=True, skip_group_check=True)

        ps_dca = ps_sm.tile([P + 1, PM + P], F32, tag="dca", name="dca")
        for bt in range(NBT):
            nc.tensor.matmul(ps_dca[:], lhs_s[bt][:], rhs_s[bt][:],
                             start=(bt == 0), stop=(bt == NBT - 1))
        dca_sb = work.tile([P + 1, PM + P], F32, tag="dca_sb", name="dca_sb")
        nc.scalar.copy(dca_sb[:], ps_dca[:])

        cc_in, _ = tc.tile([P + 1, PM + P], F32, space="DRAM", name="cc_in")
        cc_out, _ = tc.tile([P + 1, PM + P], F32, space="DRAM", addr_space="Shared", name="cc_out")
        nc.sync.dma_start(cc_in[:], dca_sb[:])
        if OPTS["collective"]:
            nc.gpsimd.collective_compute(
                "AllReduce", OP.add, replica_groups=[list(range(NC))],
                ins=[cc_in[:].opt()], outs=[cc_out[:].opt()])
        else:
            nc.sync.dma_start(cc_out[:], cc_in[:])

        # ---------------- post-reduce small math (partition 0) ----------------
        DAv = work.tile([1, PM + P], F32, tag="DAv", name="DAv")
        nc.sync.dma_start(DAv[:], cc_out[P:P + 1, 0:PM + P])
        Dv = DAv[:, 0:PM]
        Av = DAv[:, PM:PM + P]
        Cv = work.tile([1, PM], F32, tag="Cv", name="Cv")
        cc_ap = cc_out[:]
        diag = bass.AP(tensor=cc_ap.tensor, offset=cc_ap.offset,
                       ap=[[0, 1], [PM + P + M, P], [1, M]])
        nc.scalar.dma_start(Cv[:].rearrange("a (p m) -> a p m", m=M), diag)

        # msthr is pre-scaled by B on the host, ginh by 1/(1024*P*B)
        mask = work.tile([1, PM], F32, tag="mask", name="mask")
        nc.vector.tensor_tensor(
            mask[:].rearrange("a (p m) -> a p m", m=M),
            Dv.rearrange("a (p m) -> a p m", m=M),
            msthr_sb[:, :, None].broadcast_to([1, P, M]), OP.is_gt)
        tv = work.tile([1, PM], F32, tag="tv", name="tv")
        nc.vector.tensor_mul(tv[:], mask[:], Cv[:])
        tsum = work.tile([1, 1], F32, tag="tsum", name="tsum")
        nc.vector.tensor_reduce(tsum[:], tv[:], AX.X, OP.add)
        asum = work.tile([1, 1], F32, tag="asum", name="asum")
        nc.vector.tensor_reduce(asum[:], Av, AX.X, OP.add)
        u = work.tile([1, 1], F32, tag="u", name="u")
        nc.vector.scalar_tensor_tensor(u[:], tsum[:], -0.9, asum[:], OP.mult, OP.add)
        inh = work.tile([1, 1], F32, tag="inh", name="inh")
        nc.vector.tensor_mul(inh[:], u[:], ginh_sb[:])
        inh8m = work.tile([1, 1], F32, tag="inh8m", name="inh8m")
        nc.vector.tensor_scalar(inh8m[:], inh[:], -float(P), None, OP.mult)
        msum = work.tile([1, 1], F32, tag="msum", name="msum")
        nc.vector.tensor_reduce(msum[:], mask[:], AX.X, OP.add)

        # fold -P*inh into column P of each cf psum (K=1 rank-1 update), then
        # off = (sum_p cf + -P*inh) / P = mean_p cf - inh
        offs = []
        if not OPTS["slow"]:
            for bt in range(NBT):
                nc.tensor.matmul(cfps[bt][:, P:P + 1], onesc[:], inh8m[:],
                                 start=True, stop=True, skip_group_check=True)
                csum = work.tile([128, 1], F32, tag=f"csum{bt}", name=f"csum{bt}")
                nc.vector.tensor_reduce(csum[:], cfps[bt][:, 0:P + 1], AX.X, OP.add)
                off = work.tile([128, 1], F32, tag=f"off{bt}", name=f"off{bt}")
                nc.vector.tensor_scalar(off[:], csum[:], 1.0 / P, None, OP.mult)
                offs.append(off)

        # broadcast [w | inh | msum] for the (statistically never-taken)
        # microsleep slow path; not on the fast path's critical chain
        wv = work.tile([1, PM], F32, tag="wv", name="wv")
        nc.vector.tensor_scalar(wv[:], mask[:], -0.9, 1.0, OP.mult, OP.add)
        wvi = work.tile([1, PM + 2], F32, tag="wvi", name="wvi")
        nc.vector.tensor_copy(wvi[:, 0:PM], wv[:])
        nc.vector.tensor_copy(wvi[:, PM:PM + 1], inh[:])
        nc.vector.tensor_copy(wvi[:, PM + 1:PM + 2], msum[:])
        nc.scalar.dma_start(t["out_dbg"].ap(), wvi[:])
        if OPTS["slow"]:
            wvi_d, _ = tc.tile([1, PM + 2], F32, space="DRAM", name="wvi_d")
            nc.sync.dma_start(wvi_d[:], wvi[:])
            wvib = work.tile([128, PM + 2], F32, tag="wvib", name="wvib")
            nc.sync.dma_start(wvib[:], wvi_d[:].to_broadcast([128, PM + 2]))

        # ---------------- final combine ----------------
        # Fast path (mask == 0): mixed-sum is the exact PE spike sum and
        # f == sum_m scores.  Slow path recomputes everything f-dependent.
        t2s = [[work.tile([128, 512], F32, tag=f"t2_{bt}{nh}", name=f"t2_{bt}{nh}")
                for nh in range(NNH)] for bt in range(NBT)]

        def _slow_t2():
            cpb = work.tile([128, PM], F32, tag="cpb", name="cpb")
            cp_ap = t["coupTs_d"].ap()
            nc.sync.dma_start(cpb[:], bass.AP(tensor=cp_ap.tensor, offset=cp_ap.offset,
                                              ap=[[0, 128], [1, PM]]))
            inhbc = wvib[:, PM:PM + 1]
            wbc = wvib[:, 0:PM]
            for bt in range(NBT):
                u1 = work.tile([128, PM], F32, tag="u1", name="u1")
                nc.vector.tensor_mul(u1[:], scores[bt][:], wbc)
                fw = work.tile([128, P], F32, tag="fw", name="fw")
                nc.vector.tensor_reduce(fw[:], u1[:].rearrange("q (p m) -> q p m", m=M),
                                        AX.X, OP.add)
                mw = work.tile([128, P], F32, tag="mw", name="mw")
                nc.vector.tensor_mul(mw[:], fw[:], Qc[bt][:])
                # cf on the DVE: cf[b,p] = sum_p' mw[b,p'] * coupTs[p',p]
                cfw = work.tile([128, P], F32, tag="cfw", name="cfw")
                nc.vector.tensor_scalar(cfw[:], cpb[:, 0:P], mw[:, 0:1], None, OP.mult)
                for pp in range(1, P):
                    nc.vector.scalar_tensor_tensor(
                        cfw[:], cpb[:, pp * P:(pp + 1) * P], mw[:, pp:pp + 1],
                        cfw[:], OP.mult, OP.add)
                csw = work.tile([128, 1], F32, tag="csw", name="csw")
                nc.vector.tensor_reduce(csw[:], cfw[:], AX.X, OP.add)
                ofw = work.tile([128, 1], F32, tag="ofw", name="ofw")
                nc.vector.scalar_tensor_tensor(ofw[:], csw[:], 1.0 / P, inhbc,
                                               OP.mult, OP.subtract)
                for nh in range(NNH):
                    sl = slice(nh * 512, (nh + 1) * 512)
                    acc = trans.tile([128, 512], F32, tag="acc", name="acc", bufs=2)
                    nc.vector.tensor_scalar(acc[:], spk[bt][0][:, sl], fw[:, 0:1],
                                            None, OP.mult)
                    for p in range(1, P):
                        nc.vector.scalar_tensor_tensor(acc[:], spk[bt][p][:, sl],
                                                       fw[:, p:p + 1], acc[:],
                                                       OP.mult, OP.add)
                    nc.vector.tensor_scalar(t2s[bt][nh][:], acc[:], 1.0 / P, ofw[:],
                                            OP.mult, OP.add)

        def _fast_t2():
            for bt in range(NBT):
                for nh in range(NNH):
                    nc.vector.tensor_scalar(t2s[bt][nh][:], accps[bt][nh][:], 1.0 / P,
                                            offs[bt][:], OP.mult, OP.add)

        if OPTS["slow"]:
            _slow_t2()
        else:
            _fast_t2()

        for bt in range(NBT):
            for nh in range(NNH):
                sl = slice(nh * 512, (nh + 1) * 512)
                fin = trans.tile([128, 512], F32, tag="fin", name="fin", bufs=2)
                nc.vector.tensor_tensor(fin[:], t2s[bt][nh][:], rg[bt][:, sl], OP.add)
                if OPTS["final_dma"]:
                    nc.sync.dma_start(t["out_final"].ap()[bt * 128:(bt + 1) * 128, sl], fin[:])


_CACHE = {}


def _get_compiled(use_bias, use_bsel):
    key = (use_bias, use_bsel, tuple(sorted(OPTS.items())))
    if key not in _CACHE:
        nc = bacc.Bacc("TRN2", target_bir_lowering=False, debug=False,
                       enable_asserts=False, num_devices=NC)
        _emit(nc, use_bias, use_bsel)
        nc.compile()
        _CACHE[key] = nc
    return _CACHE[key]


def _prep_inputs(x, W_lin, b_lin, Wsel, bsel, Wf, spring_k, damping_c, ms_thr,
                 coupling, global_inh):
    f32 = lambda a: np.ascontiguousarray(np.asarray(a, dtype=np.float32))
    x = f32(x)
    shared = {
        "wlT": f32(np.asarray(W_lin, dtype=np.float32).T),
        "wselT": f32(np.asarray(Wsel, dtype=np.float32).reshape(PM, N).T),
        "wf": f32(np.asarray(Wf, dtype=np.float32).reshape(PM, N)),
        "kk": f32(np.asarray(spring_k) * (0.5 * DT * DT)).reshape(1, P),
        "msthr": (f32(ms_thr) * np.float32(B)).reshape(1, P),  # compare D > thr*B (exact pow2 scale)
        "coupTs": f32(np.asarray(coupling, dtype=np.float32).T / 1024.0),
        "ginh": (f32(global_inh) / np.float32(1024.0 * P * B)).reshape(1, 1),  # exact pow2 scale
    }
    use_bias = bool(np.any(np.asarray(b_lin)))
    use_bsel = bool(np.any(np.asarray(bsel)))
    if use_bias:
        shared["blin"] = f32(b_lin).reshape(1, N)
    if use_bsel:
        shared["bselv"] = f32(bsel).reshape(1, PM)
    in_maps = []
    for c in range(NC):
        m = dict(shared)
        m["x"] = np.ascontiguousarray(x[c * BL:(c + 1) * BL])
        in_maps.append(m)
    return in_maps, use_bias, use_bsel


def kernel(**inputs):
    in_maps, use_bias, use_bsel = _prep_inputs(**inputs)
    nc = _get_compiled(use_bias, use_bsel)
    res = bass_utils.run_bass_kernel_spmd(nc, in_maps, core_ids=list(range(NC)))
    if not OPTS["slow"] and float(res.results[0]["out_dbg"][0, PM + 1]) > 0.0:
        # A mode microsleeps (avg score > threshold): rerun with the exact
        # f-weighted variant.  Unreachable for softmax-of-random-logit inputs;
        # kept for full semantic coverage.
        saved = dict(OPTS)
        try:
            OPTS["slow"] = True
            nc = _get_compiled(use_bias, use_bsel)
            res = bass_utils.run_bass_kernel_spmd(nc, in_maps, core_ids=list(range(NC)))
        finally:
            OPTS.clear(); OPTS.update(saved)
    final = np.empty((B, N), np.float32)
    regmem = np.empty((B, N), np.float32)
    newx = np.empty((P, B, N), np.float32)
    for c in range(NC):
        r = res.results[c]
        final[c * BL:(c + 1) * BL] = r["out_final"]
        regmem[c * BL:(c + 1) * BL] = r["out_regmem"]
        newx[:, c * BL:(c + 1) * BL] = r["out_newx"]
    poly_mems = np.broadcast_to(newx[:, None], (P, M, B, N))
    return final, regmem, poly_mems
